# revision 12
# baseline (speedup 1.0000x reference)
"""MinGRU Trainium2 kernel.

Problem: nn_MinGRU (B=8, T=4096, D=1024, fp32)
    k  = h @ W_z.T + b_z
    th = h @ W_h.T + b_h
    z = sigmoid(k);  a = 1-z = sigmoid(-k);  b = z*g(th)
    g(x) = max(x + 0.5, sigmoid(x))
    h[t] = a[t]*h[t-1] + b[t]   (fp32-state tensor_tensor_scan)

Sharding: data-parallel over batch — core i processes sample i ([T, D]).

v3 dataflow: host pre-casts h/W to bf16, pre-swizzles weights into
per-e-tile SBUF-layout blocks (per-partition-contiguous DMAs), and
precomputes all four bias vectors. The PE runs matmuls ONLY. The output is
stored in [D, T] layout straight from the scan's [e, t] tiles (no output
transpose anywhere on device) and the host un-transposes/upcasts. Engine
assignment:
  PE:     2x8 accumulating matmuls per (chunk, e-tile)
  Scalar: a = sigmoid(-(k+bz)) (scale=-1), z = sigmoid(k+bz), s = sigmoid(th+bh)
          + weight loads and output stores (HWDGE queue; SWDGE drains slowly)
  Vector: g = max(th+bh+0.5, s), b = z*g, fp32-state scan -> hb (bf16)
  Sync:   input [t,d]->[d,t] DMA-xbar transposes (HWDGE)
  GpSimd: h-chunk loads + bias load (SWDGE queue, no compute)
Weight DMAs are consolidated to 6 (e0/e1 singles + two batched e2..7 loads)
to limit startup semaphore-epoch pressure, which serialized v2/v3 startups.
"""

import contextlib
import numpy as np
import ml_dtypes
import concourse.bass as bass
import concourse.bacc as bacc
import concourse.mybir as mybir
import concourse.tile as tile
from concourse.bass_utils import run_bass_kernel_spmd

F32 = mybir.dt.float32
BF16 = mybir.dt.bfloat16
AF = mybir.ActivationFunctionType
OP = mybir.AluOpType

B, T, D = 8, 4096, 1024
NC_CORES = 8
TC = 512                 # time chunk (one fp32 PSUM bank)
NCHUNK = T // TC         # 8
NE = D // 128            # 8 e-tiles
ND = D // 128            # 8 d-tiles
NTB = TC // 128          # 4 t-blocks per chunk


def build_program():
    nc = bacc.Bacc("TRN2", target_bir_lowering=False, debug=False)
    h_d = nc.dram_tensor("h", [T, D], BF16, kind="ExternalInput").ap()
    # weights pre-swizzled on host: [NE, 128(dp), ND, 128(e)]
    wz_d = nc.dram_tensor("wz", [NE, 128, ND, 128], BF16,
                          kind="ExternalInput").ap()
    wh_d = nc.dram_tensor("wh", [NE, 128, ND, 128], BF16,
                          kind="ExternalInput").ap()
    # biases, host-precomputed: [bz, -bz, bh, bh+0.5] each [128, NE]
    bias_d = nc.dram_tensor("bias", [128, 4 * NE], F32,
                            kind="ExternalInput").ap()
    out_d = nc.dram_tensor("out", [D, T], BF16, kind="ExternalOutput").ap()

    with tile.TileContext(nc) as tc, contextlib.ExitStack() as ctx:
        const = ctx.enter_context(tc.tile_pool(name="const", bufs=1))
        hnatp = ctx.enter_context(tc.tile_pool(name="hnat", bufs=2))
        hTp = ctx.enter_context(tc.tile_pool(name="hT", bufs=3))
        mmps = ctx.enter_context(tc.tile_pool(name="mmps", bufs=3, space="PSUM"))
        ew = ctx.enter_context(tc.tile_pool(name="ew", bufs=2))
        hbp = ctx.enter_context(tc.tile_pool(name="hb", bufs=2))

        hT_tiles = {}

        def load_chunk(ci, split_transposes=False):
            # plain bf16 loads (4 t-blocks, gpsimd/SWDGE queue) then DMA-xbar
            # transposes into [d, t] layout (sync queue; chunk 0 splits them
            # across sync+scalar to halve startup latency)
            h_nat = hnatp.tile([128, NTB, D], BF16, name=f"h_nat{ci}",
                               tag="h_nat")
            for tb in range(NTB):
                hsrc = bass.AP(
                    tensor=h_d.tensor,
                    offset=h_d.offset + (ci * TC + tb * 128) * D,
                    ap=[[D, 128], [1, D]],
                )
                nc.gpsimd.dma_start(h_nat[:, tb, :], hsrc)
            hT = hTp.tile([128, ND, TC], BF16, name=f"hT{ci}", tag="hT")
            for tb in range(NTB):
                eng = nc.scalar if (split_transposes and tb >= 2) else nc.sync
                eng.dma_start(
                    hT[:, :, tb * 128:(tb + 1) * 128],
                    h_nat[:, tb, :],
                    transpose=True,
                )
            hT_tiles[ci] = hT

        # chunk-0 h first (gates the first matmul); weights wz0/wh0 lead the
        # scalar queue so e-tile 0 can start the moment hT(c0) lands; the
        # remaining weights stream in two consolidated DMAs
        wz_t, wh_t = {}, {}

        def load_w_single(e):
            for pfx, dct, src in (("wz", wz_t, wz_d), ("wh", wh_t, wh_d)):
                w_sb = const.tile([128, ND, 128], BF16,
                                  name=f"{pfx}{e}", tag=f"{pfx}{e}")
                wsrc = bass.AP(
                    tensor=src.tensor,
                    offset=src.offset + e * 128 * ND * 128,
                    ap=[[ND * 128, 128], [128, ND], [1, 128]],
                )
                nc.scalar.dma_start(w_sb, wsrc)
                dct[e] = w_sb

        load_w_single(0)
        load_chunk(0, split_transposes=True)
        for e in range(1, NE):
            load_w_single(e)

        bias_sb = const.tile([128, 4 * NE], F32)
        nc.gpsimd.dma_start(bias_sb, bias_d)
        bz_sb = bias_sb[:, 0:NE]
        negbz = bias_sb[:, NE:2 * NE]
        bh_sb = bias_sb[:, 2 * NE:3 * NE]
        bh05 = bias_sb[:, 3 * NE:4 * NE]

        load_chunk(1)

        prev_hb = [None] * NE

        for tci in range(NCHUNK):
            hT = hT_tiles.pop(tci)

            for e in range(NE):
                k_ps = mmps.tile([128, TC], F32, name=f"k{tci}_{e}", tag="k")
                th_ps = mmps.tile([128, TC], F32, name=f"th{tci}_{e}", tag="th")
                for d in range(ND):
                    nc.tensor.matmul(k_ps, wz_t[e][:, d, :], hT[:, d, :],
                                     start=(d == 0), stop=(d == ND - 1))
                for d in range(ND):
                    nc.tensor.matmul(th_ps, wh_t[e][:, d, :], hT[:, d, :],
                                     start=(d == 0), stop=(d == ND - 1))

                # a = sigmoid(-(k+bz)); z = sigmoid(k+bz); s = sigmoid(th+bh)
                a_t = ew.tile([128, TC], F32, name=f"a{tci}_{e}", tag="a")
                z_t = ew.tile([128, TC], F32, name=f"z{tci}_{e}", tag="z")
                s_t = ew.tile([128, TC], F32, name=f"s{tci}_{e}", tag="s")
                nc.scalar.activation(a_t, k_ps, AF.Sigmoid,
                                     bias=negbz[:, e:e + 1], scale=-1.0)
                nc.scalar.activation(z_t, k_ps, AF.Sigmoid,
                                     bias=bz_sb[:, e:e + 1])
                nc.scalar.activation(s_t, th_ps, AF.Sigmoid,
                                     bias=bh_sb[:, e:e + 1])
                # g = max(th + bh + 0.5, s)
                g_t = ew.tile([128, TC], F32, name=f"g{tci}_{e}", tag="g")
                nc.vector.scalar_tensor_tensor(g_t, th_ps, bh05[:, e:e + 1],
                                               s_t, op0=OP.add, op1=OP.max)
                # b = z * g
                b_t = ew.tile([128, TC], F32, name=f"b{tci}_{e}", tag="b")
                nc.vector.tensor_tensor(b_t, z_t, g_t, OP.mult)
                # h[t] = a[t]*h[t-1] + b[t]; fp32 state, bf16 output
                hb = hbp.tile([128, TC], BF16, name=f"hb{tci}_{e}", tag=f"hb{e}")
                init = 0.0 if tci == 0 else prev_hb[e][:, TC - 1:TC]
                nc.vector.tensor_tensor_scan(hb, a_t, b_t, init,
                                             OP.mult, OP.add)
                prev_hb[e] = hb
                # store [e, t] tile directly into the [D, T] output (HWDGE,
                # scalar queue — the SWDGE queue drains slowly at kernel end)
                dst = bass.AP(
                    tensor=out_d.tensor,
                    offset=out_d.offset + e * 128 * T + tci * TC,
                    ap=[[T, 128], [1, TC]],
                )
                nc.scalar.dma_start(dst, hb)

            if tci + 2 < NCHUNK:
                load_chunk(tci + 2)

    nc.compile()
    return nc


_nc_cache = None


def _get_program():
    global _nc_cache
    if _nc_cache is None:
        _nc_cache = build_program()
    return _nc_cache


def _make_in_maps(h_prev_layer, W_z, b_z, W_h, b_h):
    bf = ml_dtypes.bfloat16
    # lhsT layout [d, e], swizzled to [NE, 128(dp), ND, 128(e)] bf16 blocks
    # that are per-partition contiguous in DRAM
    def swizzle(W):
        wT = np.ascontiguousarray(W.T.astype(np.float32))  # [d, e]
        return np.ascontiguousarray(
            wT.reshape(ND, 128, NE, 128).transpose(2, 1, 0, 3).astype(bf))

    wz8 = swizzle(W_z)
    wh8 = swizzle(W_h)
    bz8 = b_z.reshape(NE, 128).T.astype(np.float32)
    bh8 = b_h.reshape(NE, 128).T.astype(np.float32)
    bias = np.ascontiguousarray(
        np.concatenate([bz8, -bz8, bh8, bh8 + 0.5], axis=1))
    return [
        {
            "h": np.ascontiguousarray(h_prev_layer[i].astype(bf)),
            "wz": wz8, "wh": wh8, "bias": bias,
        }
        for i in range(B)
    ]


def run(inputs, trace=False, **kw):
    nc = _get_program()
    in_maps = _make_in_maps(**inputs)
    res = run_bass_kernel_spmd(nc, in_maps, core_ids=list(range(NC_CORES)),
                               trace=trace, **kw)
    # device output is [D, T] bf16; un-transpose + upcast on host
    out = np.stack([res.results[i]["out"].T.astype(np.float32)
                    for i in range(NC_CORES)], axis=0)
    return out, res


def kernel(h_prev_layer, W_z, b_z, W_h, b_h):
    out, _ = run(dict(h_prev_layer=h_prev_layer, W_z=W_z, b_z=b_z,
                      W_h=W_h, b_h=b_h))
    return out


# revision 13
# speedup vs baseline: 1.5249x; 1.5249x over previous
"""MinGRU Trainium2 kernel.

Problem: nn_MinGRU (B=8, T=4096, D=1024, fp32)
    k  = h @ W_z.T + b_z
    th = h @ W_h.T + b_h
    z = sigmoid(k);  a = 1-z = sigmoid(-k);  b = z*g(th)
    g(x) = max(x + 0.5, sigmoid(x))
    h[t] = a[t]*h[t-1] + b[t]   (fp32-state tensor_tensor_scan)

Sharding: data-parallel over batch — core i processes sample i ([T, D]).

v3 dataflow: host pre-casts h/W to bf16, pre-swizzles weights into
per-e-tile SBUF-layout blocks (per-partition-contiguous DMAs), and
precomputes all four bias vectors. The PE runs matmuls ONLY. The output is
stored in [D, T] layout straight from the scan's [e, t] tiles (no output
transpose anywhere on device) and the host un-transposes/upcasts. Engine
assignment:
  PE:     2x8 accumulating matmuls per (chunk, e-tile)
  Scalar: a = sigmoid(-(k+bz)) (scale=-1), z = sigmoid(k+bz), s = sigmoid(th+bh)
          + weight loads and output stores (HWDGE queue; SWDGE drains slowly)
  Vector: g = max(th+bh+0.5, s), b = z*g, fp32-state scan -> hb (bf16)
  Sync:   input [t,d]->[d,t] DMA-xbar transposes (HWDGE)
  GpSimd: h-chunk loads + bias load (SWDGE queue, no compute)
Weight DMAs are consolidated to 6 (e0/e1 singles + two batched e2..7 loads)
to limit startup semaphore-epoch pressure, which serialized v2/v3 startups.
"""

import contextlib
import numpy as np
import ml_dtypes
import concourse.bass as bass
import concourse.bacc as bacc
import concourse.mybir as mybir
import concourse.tile as tile
from concourse.bass_utils import run_bass_kernel_spmd

F32 = mybir.dt.float32
BF16 = mybir.dt.bfloat16
AF = mybir.ActivationFunctionType
OP = mybir.AluOpType

B, T, D = 8, 4096, 1024
NC_CORES = 8
TC = 512                 # time chunk (one fp32 PSUM bank)
NCHUNK = T // TC         # 8
NE = D // 128            # 8 e-tiles
ND = D // 128            # 8 d-tiles
NTB = TC // 128          # 4 t-blocks per chunk


def build_program():
    nc = bacc.Bacc("TRN2", target_bir_lowering=False, debug=False)
    h_d = nc.dram_tensor("h", [T, D], BF16, kind="ExternalInput").ap()
    # weights pre-swizzled on host to the SBUF layout [128(dp), ND, D(e)]
    wz_d = nc.dram_tensor("wz", [128, ND, D], BF16, kind="ExternalInput").ap()
    wh_d = nc.dram_tensor("wh", [128, ND, D], BF16, kind="ExternalInput").ap()
    # biases, host-precomputed: [bz, -bz, bh, bh+0.5] each [128, NE]
    bias_d = nc.dram_tensor("bias", [128, 4 * NE], F32,
                            kind="ExternalInput").ap()
    out_d = nc.dram_tensor("out", [D, T], BF16, kind="ExternalOutput").ap()

    with tile.TileContext(nc) as tc, contextlib.ExitStack() as ctx:
        const = ctx.enter_context(tc.tile_pool(name="const", bufs=1))
        hnatp = ctx.enter_context(tc.tile_pool(name="hnat", bufs=2))
        hTp = ctx.enter_context(tc.tile_pool(name="hT", bufs=3))
        mmps = ctx.enter_context(tc.tile_pool(name="mmps", bufs=3, space="PSUM"))
        ew = ctx.enter_context(tc.tile_pool(name="ew", bufs=2))
        hbp = ctx.enter_context(tc.tile_pool(name="hb", bufs=2))

        hT_tiles = {}

        def load_chunk(ci):
            # plain bf16 loads (4 t-blocks, gpsimd/SWDGE queue) then DMA-xbar
            # transposes into [d, t] layout (sync queue)
            h_nat = hnatp.tile([128, NTB, D], BF16, name=f"h_nat{ci}",
                               tag="h_nat")
            for tb in range(NTB):
                hsrc = bass.AP(
                    tensor=h_d.tensor,
                    offset=h_d.offset + (ci * TC + tb * 128) * D,
                    ap=[[D, 128], [1, D]],
                )
                nc.gpsimd.dma_start(h_nat[:, tb, :], hsrc)
            hT = hTp.tile([128, ND, TC], BF16, name=f"hT{ci}", tag="hT")
            for tb in range(NTB):
                nc.sync.dma_start(
                    hT[:, :, tb * 128:(tb + 1) * 128],
                    h_nat[:, tb, :],
                    transpose=True,
                )
            hT_tiles[ci] = hT

        # chunk-0 h first (gates the first matmul); each weight matrix is one
        # fully per-partition-linear DMA on the scalar queue
        load_chunk(0)
        wz_sb = const.tile([128, ND, D], BF16, name="wz_sb", tag="wz_sb")
        wh_sb = const.tile([128, ND, D], BF16, name="wh_sb", tag="wh_sb")
        nc.scalar.dma_start(wz_sb, wz_d)
        nc.scalar.dma_start(wh_sb, wh_d)

        bias_sb = const.tile([128, 4 * NE], F32)
        nc.gpsimd.dma_start(bias_sb, bias_d)
        bz_sb = bias_sb[:, 0:NE]
        negbz = bias_sb[:, NE:2 * NE]
        bh_sb = bias_sb[:, 2 * NE:3 * NE]
        bh05 = bias_sb[:, 3 * NE:4 * NE]

        load_chunk(1)

        prev_hb = [None] * NE

        for tci in range(NCHUNK):
            hT = hT_tiles.pop(tci)
            if tci + 2 < NCHUNK:
                load_chunk(tci + 2)

            for e in range(NE):
                es = slice(e * 128, (e + 1) * 128)
                k_ps = mmps.tile([128, TC], F32, name=f"k{tci}_{e}", tag="k")
                th_ps = mmps.tile([128, TC], F32, name=f"th{tci}_{e}", tag="th")
                for d in range(ND):
                    nc.tensor.matmul(k_ps, wz_sb[:, d, es], hT[:, d, :],
                                     start=(d == 0), stop=(d == ND - 1))
                for d in range(ND):
                    nc.tensor.matmul(th_ps, wh_sb[:, d, es], hT[:, d, :],
                                     start=(d == 0), stop=(d == ND - 1))

                # a = sigmoid(-(k+bz)); z = sigmoid(k+bz); s = sigmoid(th+bh)
                a_t = ew.tile([128, TC], F32, name=f"a{tci}_{e}", tag="a")
                z_t = ew.tile([128, TC], F32, name=f"z{tci}_{e}", tag="z")
                s_t = ew.tile([128, TC], F32, name=f"s{tci}_{e}", tag="s")
                nc.scalar.activation(a_t, k_ps, AF.Sigmoid,
                                     bias=negbz[:, e:e + 1], scale=-1.0)
                nc.scalar.activation(z_t, k_ps, AF.Sigmoid,
                                     bias=bz_sb[:, e:e + 1])
                nc.scalar.activation(s_t, th_ps, AF.Sigmoid,
                                     bias=bh_sb[:, e:e + 1])
                # g = max(th + bh + 0.5, s)
                g_t = ew.tile([128, TC], F32, name=f"g{tci}_{e}", tag="g")
                nc.vector.scalar_tensor_tensor(g_t, th_ps, bh05[:, e:e + 1],
                                               s_t, op0=OP.add, op1=OP.max)
                # b = z * g
                b_t = ew.tile([128, TC], F32, name=f"b{tci}_{e}", tag="b")
                nc.vector.tensor_tensor(b_t, z_t, g_t, OP.mult)
                # h[t] = a[t]*h[t-1] + b[t]; fp32 state, bf16 output
                hb = hbp.tile([128, TC], BF16, name=f"hb{tci}_{e}", tag=f"hb{e}")
                init = 0.0 if tci == 0 else prev_hb[e][:, TC - 1:TC]
                nc.vector.tensor_tensor_scan(hb, a_t, b_t, init,
                                             OP.mult, OP.add)
                prev_hb[e] = hb
                # store [e, t] tile directly into the [D, T] output (HWDGE,
                # sync queue — the SWDGE queue drains slowly at kernel end)
                dst = bass.AP(
                    tensor=out_d.tensor,
                    offset=out_d.offset + e * 128 * T + tci * TC,
                    ap=[[T, 128], [1, TC]],
                )
                nc.sync.dma_start(dst, hb)

    nc.compile()
    return nc


_nc_cache = None


def _get_program():
    global _nc_cache
    if _nc_cache is None:
        _nc_cache = build_program()
    return _nc_cache


def _make_in_maps(h_prev_layer, W_z, b_z, W_h, b_h):
    bf = ml_dtypes.bfloat16
    # lhsT layout [d, e], swizzled to [NE, 128(dp), ND, 128(e)] bf16 blocks
    # that are per-partition contiguous in DRAM
    def swizzle(W):
        wT = np.ascontiguousarray(W.T.astype(np.float32))  # [d, e]
        return np.ascontiguousarray(
            wT.reshape(ND, 128, D).transpose(1, 0, 2).astype(bf))

    wz8 = swizzle(W_z)
    wh8 = swizzle(W_h)
    bz8 = b_z.reshape(NE, 128).T.astype(np.float32)
    bh8 = b_h.reshape(NE, 128).T.astype(np.float32)
    bias = np.ascontiguousarray(
        np.concatenate([bz8, -bz8, bh8, bh8 + 0.5], axis=1))
    return [
        {
            "h": np.ascontiguousarray(h_prev_layer[i].astype(bf)),
            "wz": wz8, "wh": wh8, "bias": bias,
        }
        for i in range(B)
    ]


def run(inputs, trace=False, **kw):
    nc = _get_program()
    in_maps = _make_in_maps(**inputs)
    res = run_bass_kernel_spmd(nc, in_maps, core_ids=list(range(NC_CORES)),
                               trace=trace, **kw)
    # device output is [D, T] bf16; un-transpose + upcast on host
    out = np.stack([res.results[i]["out"].T.astype(np.float32)
                    for i in range(NC_CORES)], axis=0)
    return out, res


def kernel(h_prev_layer, W_z, b_z, W_h, b_h):
    out, _ = run(dict(h_prev_layer=h_prev_layer, W_z=W_z, b_z=b_z,
                      W_h=W_h, b_h=b_h))
    return out


# revision 14
# speedup vs baseline: 1.5371x; 1.0080x over previous
"""MinGRU Trainium2 kernel.

Problem: nn_MinGRU (B=8, T=4096, D=1024, fp32)
    k  = h @ W_z.T + b_z
    th = h @ W_h.T + b_h
    z = sigmoid(k);  a = 1-z = sigmoid(-k);  b = z*g(th)
    g(x) = max(x + 0.5, sigmoid(x))
    h[t] = a[t]*h[t-1] + b[t]   (fp32-state tensor_tensor_scan)

Sharding: data-parallel over batch — core i processes sample i ([T, D]).

v3 dataflow: host pre-casts h/W to bf16, pre-swizzles weights into
per-e-tile SBUF-layout blocks (per-partition-contiguous DMAs), and
precomputes all four bias vectors. The PE runs matmuls ONLY. The output is
stored in [D, T] layout straight from the scan's [e, t] tiles (no output
transpose anywhere on device) and the host un-transposes/upcasts. Engine
assignment:
  PE:     2x8 accumulating matmuls per (chunk, e-tile)
  Scalar: a = sigmoid(-(k+bz)) (scale=-1), z = sigmoid(k+bz), s = sigmoid(th+bh)
          + weight loads and output stores (HWDGE queue; SWDGE drains slowly)
  Vector: g = max(th+bh+0.5, s), b = z*g, fp32-state scan -> hb (bf16)
  Sync:   input [t,d]->[d,t] DMA-xbar transposes (HWDGE)
  GpSimd: h-chunk loads + bias load (SWDGE queue, no compute)
Weight DMAs are consolidated to 6 (e0/e1 singles + two batched e2..7 loads)
to limit startup semaphore-epoch pressure, which serialized v2/v3 startups.
"""

import contextlib
import numpy as np
import ml_dtypes
import concourse.bass as bass
import concourse.bacc as bacc
import concourse.mybir as mybir
import concourse.tile as tile
from concourse.bass_utils import run_bass_kernel_spmd

F32 = mybir.dt.float32
BF16 = mybir.dt.bfloat16
AF = mybir.ActivationFunctionType
OP = mybir.AluOpType

B, T, D = 8, 4096, 1024
NC_CORES = 8
TC = 512                 # time chunk (one fp32 PSUM bank)
NCHUNK = T // TC         # 8
NE = D // 128            # 8 e-tiles
ND = D // 128            # 8 d-tiles
NTB = TC // 128          # 4 t-blocks per chunk


def build_program():
    nc = bacc.Bacc("TRN2", target_bir_lowering=False, debug=False)
    h_d = nc.dram_tensor("h", [T, D], BF16, kind="ExternalInput").ap()
    # weights pre-swizzled on host to the SBUF layout [128(dp), ND, D(e)]
    wz_d = nc.dram_tensor("wz", [128, ND, D], BF16, kind="ExternalInput").ap()
    wh_d = nc.dram_tensor("wh", [128, ND, D], BF16, kind="ExternalInput").ap()
    # biases, host-precomputed: [bz, -bz, bh, bh+0.5] each [128, NE]
    bias_d = nc.dram_tensor("bias", [128, 4 * NE], F32,
                            kind="ExternalInput").ap()
    out_d = nc.dram_tensor("out", [D, T], BF16, kind="ExternalOutput").ap()

    with tile.TileContext(nc) as tc, contextlib.ExitStack() as ctx:
        const = ctx.enter_context(tc.tile_pool(name="const", bufs=1))
        hnatp = ctx.enter_context(tc.tile_pool(name="hnat", bufs=2))
        hTp = ctx.enter_context(tc.tile_pool(name="hT", bufs=3))
        mmps = ctx.enter_context(tc.tile_pool(name="mmps", bufs=3, space="PSUM"))
        ew = ctx.enter_context(tc.tile_pool(name="ew", bufs=2))
        hbp = ctx.enter_context(tc.tile_pool(name="hb", bufs=2))

        hT_tiles = {}

        def load_chunk(ci):
            # plain bf16 loads (4 t-blocks, gpsimd/SWDGE queue) then DMA-xbar
            # transposes into [d, t] layout (sync queue)
            h_nat = hnatp.tile([128, NTB, D], BF16, name=f"h_nat{ci}",
                               tag="h_nat")
            for tb in range(NTB):
                hsrc = bass.AP(
                    tensor=h_d.tensor,
                    offset=h_d.offset + (ci * TC + tb * 128) * D,
                    ap=[[D, 128], [1, D]],
                )
                nc.gpsimd.dma_start(h_nat[:, tb, :], hsrc)
            hT = hTp.tile([128, ND, TC], BF16, name=f"hT{ci}", tag="hT")
            for tb in range(NTB):
                nc.sync.dma_start(
                    hT[:, :, tb * 128:(tb + 1) * 128],
                    h_nat[:, tb, :],
                    transpose=True,
                )
            hT_tiles[ci] = hT

        # chunk-0 h first — its loads must win the DMA engines or the whole
        # startup serializes behind the 4MB weight stream. Weights arrive as
        # 4 column-block DMAs per matrix (e-tile pairs, in consumption order)
        # into one big SBUF tile each.
        load_chunk(0)
        wz_sb = const.tile([128, ND, D], BF16, name="wz_sb", tag="wz_sb")
        wh_sb = const.tile([128, ND, D], BF16, name="wh_sb", tag="wh_sb")
        WBLK = D // 4

        def load_w_block(b):
            for w_sb, src in ((wz_sb, wz_d), (wh_sb, wh_d)):
                wsrc = bass.AP(
                    tensor=src.tensor,
                    offset=src.offset + b * 128 * ND * WBLK,
                    ap=[[ND * WBLK, 128], [WBLK, ND], [1, WBLK]],
                )
                nc.scalar.dma_start(
                    w_sb[:, :, b * WBLK:(b + 1) * WBLK], wsrc)

        load_w_block(0)
        load_chunk(1)
        bias_sb = const.tile([128, 4 * NE], F32)
        nc.gpsimd.dma_start(bias_sb, bias_d)
        bz_sb = bias_sb[:, 0:NE]
        negbz = bias_sb[:, NE:2 * NE]
        bh_sb = bias_sb[:, 2 * NE:3 * NE]
        bh05 = bias_sb[:, 3 * NE:4 * NE]
        for b in range(1, 4):
            load_w_block(b)

        prev_hb = [None] * NE

        for tci in range(NCHUNK):
            hT = hT_tiles.pop(tci)
            if tci + 2 < NCHUNK:
                load_chunk(tci + 2)

            for e in range(NE):
                es = slice(e * 128, (e + 1) * 128)
                k_ps = mmps.tile([128, TC], F32, name=f"k{tci}_{e}", tag="k")
                th_ps = mmps.tile([128, TC], F32, name=f"th{tci}_{e}", tag="th")
                for d in range(ND):
                    nc.tensor.matmul(k_ps, wz_sb[:, d, es], hT[:, d, :],
                                     start=(d == 0), stop=(d == ND - 1))
                for d in range(ND):
                    nc.tensor.matmul(th_ps, wh_sb[:, d, es], hT[:, d, :],
                                     start=(d == 0), stop=(d == ND - 1))

                # a = sigmoid(-(k+bz)); z = sigmoid(k+bz); s = sigmoid(th+bh)
                a_t = ew.tile([128, TC], F32, name=f"a{tci}_{e}", tag="a")
                z_t = ew.tile([128, TC], F32, name=f"z{tci}_{e}", tag="z")
                s_t = ew.tile([128, TC], F32, name=f"s{tci}_{e}", tag="s")
                nc.scalar.activation(a_t, k_ps, AF.Sigmoid,
                                     bias=negbz[:, e:e + 1], scale=-1.0)
                nc.scalar.activation(z_t, k_ps, AF.Sigmoid,
                                     bias=bz_sb[:, e:e + 1])
                nc.scalar.activation(s_t, th_ps, AF.Sigmoid,
                                     bias=bh_sb[:, e:e + 1])
                # g = max(th + bh + 0.5, s)
                g_t = ew.tile([128, TC], F32, name=f"g{tci}_{e}", tag="g")
                nc.vector.scalar_tensor_tensor(g_t, th_ps, bh05[:, e:e + 1],
                                               s_t, op0=OP.add, op1=OP.max)
                # b = z * g
                b_t = ew.tile([128, TC], F32, name=f"b{tci}_{e}", tag="b")
                nc.vector.tensor_tensor(b_t, z_t, g_t, OP.mult)
                # h[t] = a[t]*h[t-1] + b[t]; fp32 state, bf16 output
                hb = hbp.tile([128, TC], BF16, name=f"hb{tci}_{e}", tag=f"hb{e}")
                init = 0.0 if tci == 0 else prev_hb[e][:, TC - 1:TC]
                nc.vector.tensor_tensor_scan(hb, a_t, b_t, init,
                                             OP.mult, OP.add)
                prev_hb[e] = hb
                # store [e, t] tile directly into the [D, T] output (HWDGE,
                # sync queue — the SWDGE queue drains slowly at kernel end)
                dst = bass.AP(
                    tensor=out_d.tensor,
                    offset=out_d.offset + e * 128 * T + tci * TC,
                    ap=[[T, 128], [1, TC]],
                )
                nc.sync.dma_start(dst, hb)

    nc.compile()
    return nc


_nc_cache = None


def _get_program():
    global _nc_cache
    if _nc_cache is None:
        _nc_cache = build_program()
    return _nc_cache


def _make_in_maps(h_prev_layer, W_z, b_z, W_h, b_h):
    bf = ml_dtypes.bfloat16
    # lhsT layout [d, e], swizzled to [NE, 128(dp), ND, 128(e)] bf16 blocks
    # that are per-partition contiguous in DRAM
    def swizzle(W):
        wT = np.ascontiguousarray(W.T.astype(np.float32))  # [d, e]
        # [4 blocks][128 dp][ND dt][256 e] — per-partition contiguous per block
        w = wT.reshape(ND, 128, 4, 256).transpose(2, 1, 0, 3)
        return np.ascontiguousarray(w.astype(bf))

    wz8 = swizzle(W_z)
    wh8 = swizzle(W_h)
    bz8 = b_z.reshape(NE, 128).T.astype(np.float32)
    bh8 = b_h.reshape(NE, 128).T.astype(np.float32)
    bias = np.ascontiguousarray(
        np.concatenate([bz8, -bz8, bh8, bh8 + 0.5], axis=1))
    return [
        {
            "h": np.ascontiguousarray(h_prev_layer[i].astype(bf)),
            "wz": wz8, "wh": wh8, "bias": bias,
        }
        for i in range(B)
    ]


def run(inputs, trace=False, **kw):
    nc = _get_program()
    in_maps = _make_in_maps(**inputs)
    res = run_bass_kernel_spmd(nc, in_maps, core_ids=list(range(NC_CORES)),
                               trace=trace, **kw)
    # device output is [D, T] bf16; un-transpose + upcast on host
    out = np.stack([res.results[i]["out"].T.astype(np.float32)
                    for i in range(NC_CORES)], axis=0)
    return out, res


def kernel(h_prev_layer, W_z, b_z, W_h, b_h):
    out, _ = run(dict(h_prev_layer=h_prev_layer, W_z=W_z, b_z=b_z,
                      W_h=W_h, b_h=b_h))
    return out
